# revision 32
# baseline (speedup 1.0000x reference)
"""Causal segment-masked depthwise conv (K=5) + pointwise conv, 8-core SPMD.

Strategy (bf16 data path), v2:
  Host: pack each batch row's covered segments into one gap-free global
  stream (T = 32768 = 8 cores x 4096; segments exactly partition [0,L) so
  the stream is x itself), split evenly with a 4-element halo, transpose
  to [C, stream], cast bf16.  Cross-run tap leakage fixed by a sparse
  host-side correction at the first 4 positions of each run.  b_dw folded
  into b_eff = b_pw + w_pw @ b_dw.

  Device per core (4 superblocks of 1024 cols, 4 channel chunks of 128):
  depthwise conv split across engines by measured throughput:
    PE    (diag-matmul, 10 mm/unit @ ~210ns): units (0,2)(0,3)(1,2)(2,0)
          (2,2)(2,3)(3,2)
    DVE   (5 tensor_scalar + 4 tensor_tensor @ ~450/~690ns): units
          (0,0)(1,0)(1,3)(3,0)(3,3), plus tap-scales for the Pool units
    Pool  (4 tensor_tensor adds @ ~2.2us): units (0,1)(1,1)(2,1)
    ACT   tap-scales for (3,1) (adds on DVE), all PSUM evacuations + bias
  Pointwise: per sb 4 dch x [h=0 chain of 4, h=1 chain of 4] accumulating
  matmuls (sequential-h), ACT adds b_eff over 1024 cols + bf16 cast,
  2 batched stores per sb.  Host upcasts + scatters + corrects.
"""

import sys

sys.path.insert(0, "/opt/trn_rl_repo")

import numpy as np
import ml_dtypes

BF16 = ml_dtypes.bfloat16

B, L, C, K, S = 8, 4096, 512, 5, 8
NCORES = 8
CCH = C // 128          # 4 channel chunks
Q = 4096                # stream cols per core
NSB = 4                 # 1024-wide superblocks per core
SBW = 1024
SBPAD = 1032            # 4 halo + 1024 + 4 pad
BLK = 512

# conv unit assignment per (sb, chunk).  Pool is not used: its SBUF
# traffic shares the DVE port and collapses DVE tensor_scalar to ~1/5
# throughput (measured).
PE_UNITS = [(0, 2), (0, 3), (2, 2), (2, 3), (1, 2), (2, 1), (3, 2)]
ACT_UNITS = [(0, 1), (3, 1)]            # scales on ACT, adds on DVE
DVE_UNITS = [(0, 0), (1, 0), (1, 1), (1, 3), (2, 0), (3, 0), (3, 3)]
DIAG_CHUNKS = sorted({j for _, j in PE_UNITS})   # [0, 1, 2, 3]
DIAG_IDX = {j: i for i, j in enumerate(DIAG_CHUNKS)}

_cached = {}


def _build_nc():
    import concourse.mybir as mybir
    from concourse import bacc
    from concourse.tile import TileContext

    f32 = mybir.dt.float32
    bf16 = mybir.dt.bfloat16
    Alu = mybir.AluOpType
    Act = mybir.ActivationFunctionType

    nc = bacc.Bacc(num_swdge_queues=1)
    xin_d = nc.declare_dram_parameter("xin", [NSB, 128, CCH, SBPAD], bf16, isOutput=False)
    # cst: wdiag all 4 chunks (4*K cols) then beff (CCH cols), fp32
    cst_d = nc.declare_dram_parameter("cst", [128, CCH * K + CCH], f32, isOutput=False)
    diag_d = nc.declare_dram_parameter(
        "diag", [128, len(DIAG_CHUNKS), K, 128], bf16, isOutput=False
    )
    wpwt_d = nc.declare_dram_parameter("wpwt", [128, CCH, CCH, 128], bf16, isOutput=False)
    out_d = nc.declare_dram_parameter("out", [NSB, 128, CCH, SBW], bf16, isOutput=True)

    with TileContext(nc) as tc:
        with (
            tc.tile_pool(name="consts", bufs=1) as cpool,
            tc.tile_pool(name="xin", bufs=4) as xin_pool,
            tc.tile_pool(name="dwt", bufs=4) as dwt_pool,
            tc.tile_pool(name="ysc", bufs=3) as y_pool,
            tc.tile_pool(name="outsb", bufs=2) as ob_pool,
            tc.tile_pool(name="dwps", bufs=2, space="PSUM") as dw_psum,
            tc.tile_pool(name="outps", bufs=2, space="PSUM") as out_psum,
            tc.tile_pool(name="p3ps", bufs=2, space="PSUM") as sb3_psum,
        ):
            # scalar ring: cst, diag hi-chunks, xin1, diag lo-chunks, xin3
            # sync ring: xin0, xin2, wpwt (stores also go on sync later)
            cst = cpool.tile([128, CCH * K + CCH], f32)
            nc.scalar.dma_start(out=cst[:], in_=cst_d[:])
            wdiag = cst[:, 0 : CCH * K]
            beff = cst[:, CCH * K : CCH * K + CCH]
            diag = cpool.tile([128, len(DIAG_CHUNKS), K, 128], bf16)
            nc.scalar.dma_start(out=diag[:], in_=diag_d[:])

            xts = [None] * NSB
            t0 = xin_pool.tile([128, CCH, SBPAD], bf16, tag="xin", name="xin0")
            # DVE's chunks (0,1) of sb0 first so it starts earliest
            nc.sync.dma_start(out=t0[:, 0:2], in_=xin_d[0, :, 0:2])
            nc.sync.dma_start(out=t0[:, 2:4], in_=xin_d[0, :, 2:4])
            xts[0] = t0
            t1 = xin_pool.tile([128, CCH, SBPAD], bf16, tag="xin", name="xin1")
            nc.sync.dma_start(out=t1[:], in_=xin_d[1])
            xts[1] = t1
            t2 = xin_pool.tile([128, CCH, SBPAD], bf16, tag="xin", name="xin2")
            nc.scalar.dma_start(out=t2[:], in_=xin_d[2])
            xts[2] = t2
            wpwt = cpool.tile([128, CCH, CCH, 128], bf16)
            nc.sync.dma_start(out=wpwt[:], in_=wpwt_d[:])
            t3 = xin_pool.tile([128, CCH, SBPAD], bf16, tag="xin", name="xin3")
            nc.scalar.dma_start(out=t3[:], in_=xin_d[3])
            xts[3] = t3

            # PE warm-up / filler chains on a memset tile while DMA lands.
            # They use the sb3 psum tag (only needed at the very end) and
            # are never read, so rotation is WAW-only and harmless.
            warm_t = cpool.tile([128, BLK], bf16)
            nc.vector.memset(warm_t[:], 0.0)
            _warm_n = [0]

            def warm(n):
                wp = sb3_psum.tile(
                    [128, BLK], f32, tag="p3", name=f"warm{_warm_n[0]}"
                )
                _warm_n[0] += 1
                for wi in range(n):
                    nc.tensor.matmul(
                        wp[:],
                        lhsT=warm_t[:, 0:128],
                        rhs=warm_t[:],
                        start=(wi == 0),
                        stop=(wi == n - 1),
                    )

            warm(24)

            dwts = [
                [
                    dwt_pool.tile([128, SBW], bf16, tag=f"dwt{j}", name=f"dwt{j}_{sb}")
                    for j in range(CCH)
                ]
                for sb in range(NSB)
            ]

            def conv_pe(sb, j):
                xtv = xts[sb]
                jj = DIAG_IDX[j]
                for h in range(2):
                    ps = dw_psum.tile([128, BLK], f32, tag="dwps", name=f"cps{sb}_{j}_{h}")
                    for k in range(K):
                        nc.tensor.matmul(
                            ps[:],
                            lhsT=diag[:, jj, k, :],
                            rhs=xtv[:, j, h * BLK + k : h * BLK + k + BLK],
                            start=(k == 0),
                            stop=(k == K - 1),
                        )
                    nc.scalar.copy(dwts[sb][j][:, h * BLK : (h + 1) * BLK], ps[:])

            def conv_dve(sb, j):
                # dwt = w0*x[0:1024]; then += wk*x[k:k+1024] via scratch
                xtv = xts[sb]
                sl = dwts[sb][j][:]
                nc.vector.tensor_scalar_mul(
                    sl, xtv[:, j, 0:SBW], wdiag[:, j * K : j * K + 1]
                )
                y = y_pool.tile([128, SBW], bf16, tag="dy", name=f"dy{sb}_{j}")
                for k in range(1, K):
                    nc.vector.tensor_scalar_mul(
                        y[:], xtv[:, j, k : k + SBW], wdiag[:, j * K + k : j * K + k + 1]
                    )
                    nc.vector.tensor_tensor(sl, sl, y[:], op=Alu.add)

            def conv_scales_dve(sb, j):
                # 5 scaled taps into distinct scratch tiles (for Pool adds)
                xtv = xts[sb]
                ys = []
                for k in range(K):
                    y = y_pool.tile([128, SBW], bf16, tag=f"y{k}", name=f"y{k}_{sb}_{j}")
                    nc.vector.tensor_scalar_mul(
                        y[:], xtv[:, j, k : k + SBW], wdiag[:, j * K + k : j * K + k + 1]
                    )
                    ys.append(y)
                return ys

            def conv_scales_act(sb, j):
                xtv = xts[sb]
                ys = []
                for k in range(K):
                    y = y_pool.tile([128, SBW], bf16, tag=f"y{k}", name=f"ya{k}_{sb}_{j}")
                    nc.scalar.activation(
                        y[:], xtv[:, j, k : k + SBW], Act.Copy,
                        bias=0.0, scale=wdiag[:, j * K + k : j * K + k + 1],
                    )
                    ys.append(y)
                return ys

            def conv_adds(engine, sb, j, ys):
                sl = dwts[sb][j][:]
                engine.tensor_tensor(sl, ys[0][:], ys[1][:], op=Alu.add)
                for k in range(2, K):
                    engine.tensor_tensor(sl, sl, ys[k][:], op=Alu.add)

            obs = {}

            def pw_dch(sb, dch):
                dt = dwts[sb]
                if dch == 0:
                    obs[sb] = ob_pool.tile(
                        [128, CCH, SBW], bf16, tag="outsb", name=f"ob{sb}"
                    )
                ob = obs[sb]
                po = out_psum.tile(
                    [128, 2, BLK], f32, tag="outps", name=f"po{dch}_{sb}"
                )
                for h in range(2):
                    for j in range(CCH):
                        nc.tensor.matmul(
                            po[:, h, :],
                            lhsT=wpwt[:, j, dch, :],
                            rhs=dt[j][:, h * BLK : (h + 1) * BLK],
                            start=(j == 0),
                            stop=(j == CCH - 1),
                        )
                nc.scalar.add(ob[:, dch, :], po[:], beff[:, dch : dch + 1])
                # store each dch as soon as it is ready, on the sync ring
                # (idle after the input loads) so the tail drains fast
                nc.sync.dma_start(
                    out=out_d[sb, :, dch : dch + 1, :],
                    in_=ob[:, dch : dch + 1, :],
                )

            # ---- ACT scale-set for (0,1) in ACT's early idle window ----
            ys_01 = conv_scales_act(0, 1)

            # ---- DVE full units (early sbs first, (3,3) before (3,1) adds)
            conv_dve(0, 0)
            conv_dve(1, 0)
            conv_adds(nc.vector, 0, 1, ys_01)
            conv_dve(1, 1)
            conv_dve(1, 3)
            conv_dve(2, 0)
            conv_dve(3, 0)

            # ---- PE conv units, arrival-aware, with filler to absorb
            # DMA-arrival jitter without p-state resets ----
            conv_pe(0, 2)
            conv_pe(0, 3)
            warm(6)
            conv_pe(1, 2)
            warm(4)
            conv_pe(2, 2)
            conv_pe(2, 3)

            # ---- ACT scales for (3,1); DVE finishes sb3 in half-width
            # ops so the pointwise gate lands as early as possible ----
            ys_31 = conv_scales_act(3, 1)

            def conv_dve_half(sb, j, h):
                xtv = xts[sb]
                o = h * BLK
                sl = dwts[sb][j][:, o : o + BLK]
                nc.vector.tensor_scalar_mul(
                    sl, xtv[:, j, o : o + BLK], wdiag[:, j * K : j * K + 1]
                )
                y = y_pool.tile([128, BLK], bf16, tag="dyh", name=f"dyh{sb}_{j}_{h}")
                for k in range(1, K):
                    nc.vector.tensor_scalar_mul(
                        y[:], xtv[:, j, o + k : o + k + BLK],
                        wdiag[:, j * K + k : j * K + k + 1],
                    )
                    nc.vector.tensor_tensor(sl, sl, y[:], op=Alu.add)

            def conv_adds_half(sb, j, ys, h):
                o = h * BLK
                sl = dwts[sb][j][:, o : o + BLK]
                nc.vector.tensor_tensor(
                    sl, ys[0][:, o : o + BLK], ys[1][:, o : o + BLK], op=Alu.add
                )
                for k in range(2, K):
                    nc.vector.tensor_tensor(sl, sl, ys[k][:, o : o + BLK], op=Alu.add)

            conv_dve_half(3, 3, 0)
            conv_adds_half(3, 1, ys_31, 0)
            conv_dve_half(3, 3, 1)
            conv_adds_half(3, 1, ys_31, 1)

            # ---- pointwise; PE order fills DMA/DVE wait with conv units
            conv_pe(2, 1)
            conv_pe(3, 2)
            for dch in range(CCH):
                pw_dch(0, dch)
            for dch in range(CCH):
                pw_dch(1, dch)
            for dch in range(CCH):
                pw_dch(2, dch)
            # sb3 split by halves: h0 runs as soon as the h0 conv lands
            ob3 = ob_pool.tile([128, CCH, SBW], bf16, tag="outsb", name="ob3")
            dt3 = dwts[3]
            for h in range(2):
                for dch in range(CCH):
                    ph = sb3_psum.tile(
                        [128, BLK], f32, tag="p3", name=f"p3_{dch}_{h}"
                    )
                    for j in range(CCH):
                        nc.tensor.matmul(
                            ph[:],
                            lhsT=wpwt[:, j, dch, :],
                            rhs=dt3[j][:, h * BLK : (h + 1) * BLK],
                            start=(j == 0),
                            stop=(j == CCH - 1),
                        )
                    nc.scalar.add(
                        ob3[:, dch, h * BLK : (h + 1) * BLK],
                        ph[:],
                        beff[:, dch : dch + 1],
                    )
                    if h == 1:
                        nc.sync.dma_start(
                            out=out_d[3, :, dch : dch + 1, :],
                            in_=ob3[:, dch : dch + 1, :],
                        )

    nc.finalize()
    return nc


def _get_nc():
    if "nc" not in _cached:
        _cached["nc"] = _build_nc()
    return _cached["nc"]


def _analyze(segment_boundaries):
    starts = segment_boundaries[..., 0].astype(np.int64)  # [B,S]
    ends = segment_boundaries[..., 1].astype(np.int64)
    pos = np.arange(L)
    in_seg = (pos[None, None, :] >= starts[..., None]) & (
        pos[None, None, :] < ends[..., None]
    )  # [B,S,L]
    covered = in_seg.any(axis=1)
    seg_id = np.where(covered, in_seg.argmax(axis=1), -1)  # [B,L]
    return covered, seg_id


def kernel(x, segment_boundaries, w_dw, b_dw, w_pw, b_pw):
    from concourse.bass_utils import run_bass_kernel_spmd

    x = np.asarray(x, dtype=np.float32)
    sb_in = np.asarray(segment_boundaries)
    w_dw = np.asarray(w_dw, dtype=np.float32)
    b_dw = np.asarray(b_dw, dtype=np.float32)
    w_pw = np.asarray(w_pw, dtype=np.float32)
    b_pw = np.asarray(b_pw, dtype=np.float32)

    covered, seg_id = _analyze(sb_in)

    # ---- gap-free run decomposition ----
    runs = []  # (b, s, e, p0) with p0 = stream offset
    pieces = []
    src_b_parts = []
    src_l_parts = []
    p0 = 0
    for b in range(B):
        sid = seg_id[b]
        change = np.nonzero(np.diff(sid) != 0)[0] + 1
        bounds = np.concatenate([[0], change, [L]])
        for s, e in zip(bounds[:-1], bounds[1:]):
            if sid[s] < 0:
                continue
            runs.append((b, int(s), int(e), p0))
            pieces.append(x[b, s:e])
            src_b_parts.append(np.full(e - s, b, np.int64))
            src_l_parts.append(np.arange(s, e, dtype=np.int64))
            p0 += e - s
    if pieces:
        stream = np.concatenate(pieces, axis=0)
        src_b = np.concatenate(src_b_parts)
        src_l = np.concatenate(src_l_parts)
    else:
        stream = np.zeros((0, C), np.float32)
        src_b = np.zeros(0, np.int64)
        src_l = np.zeros(0, np.int64)
    T = stream.shape[0]
    Qc = -(-T // NCORES) if T else 1
    assert Qc <= Q, f"stream quota {Qc} too large"

    # ---- per-core inputs ----
    wdiag = np.ascontiguousarray(
        w_dw.reshape(CCH, 128, K).transpose(1, 0, 2).reshape(128, CCH * K)
    )
    beff_full = b_pw + w_pw @ b_dw                      # [C]
    beffr = np.ascontiguousarray(beff_full.reshape(CCH, 128).T)
    cst = np.concatenate([wdiag, beffr], axis=1).astype(np.float32)
    diag = np.zeros((128, len(DIAG_CHUNKS), K, 128), np.float32)
    for jj, j in enumerate(DIAG_CHUNKS):
        for k in range(K):
            np.fill_diagonal(diag[:, jj, k, :], w_dw[j * 128 : (j + 1) * 128, k])
    diag = diag.astype(BF16)
    wpwt = np.ascontiguousarray(
        w_pw.reshape(CCH, 128, CCH, 128).transpose(3, 2, 0, 1)
    ).astype(BF16)

    # transposed bf16 stream with 4 zero cols in front
    streamT = np.zeros((C, 4 + T), dtype=BF16)
    streamT[:, 4:] = stream.T.astype(BF16)

    in_maps = []
    spans = []
    for i in range(NCORES):
        lo, hi = i * Qc, min((i + 1) * Qc, T)
        lo = min(lo, T)
        spans.append((lo, hi))
        xin = np.zeros((NSB, 128, CCH, SBPAD), dtype=BF16)
        for sbi in range(NSB):
            a = lo + sbi * SBW            # first needed stream col minus 4
            w = min(SBW + 4, 4 + T - a)
            if w <= 0:
                continue
            blkdat = streamT[:, a : a + w]  # [C, w]
            xin[sbi, :, :, :w] = blkdat.reshape(CCH, 128, w).transpose(1, 0, 2)
        in_maps.append({"xin": xin, "cst": cst, "diag": diag, "wpwt": wpwt})

    nc = _get_nc()
    res = run_bass_kernel_spmd(nc, in_maps, list(range(NCORES)))

    # ---- gather (device out is [NSB, 128, CCH, SBW] block-packed) ----
    so_out = np.zeros((T, C), np.float32)
    for i, (lo, hi) in enumerate(spans):
        if hi > lo:
            full = (
                res.results[i]["out"]
                .astype(np.float32)
                .transpose(0, 3, 2, 1)
                .reshape(NSB * SBW, C)
            )
            so_out[lo:hi] = full[: hi - lo]
    out = np.zeros((B, L, C), np.float32)
    out[src_b, src_l] = so_out

    # ---- sparse correction at the first 4 positions of each run ----
    fix_b, fix_l, fix_delta = [], [], []
    for (b, s, e, p0r) in runs:
        n = e - s
        for q in range(min(K - 1, n)):
            l = s + q
            t = p0r + q
            acc = np.zeros(C, np.float32)
            hit = False
            for d in range(q + 1, K):
                v_dev = stream[t - d] if t - d >= 0 else None
                l2 = l - d
                v_ref = (
                    x[b, l2]
                    if (l2 >= 0 and seg_id[b, l2] == seg_id[b, l])
                    else None
                )
                if v_dev is None and v_ref is None:
                    continue
                diff = (v_ref if v_ref is not None else 0.0) - (
                    v_dev if v_dev is not None else 0.0
                )
                acc += w_dw[:, K - 1 - d] * diff
                hit = True
            if hit:
                fix_b.append(b)
                fix_l.append(l)
                fix_delta.append(acc)
    if fix_b:
        deltas = np.stack(fix_delta) @ w_pw.T
        out[np.array(fix_b), np.array(fix_l)] += deltas

    return out


# revision 33
# speedup vs baseline: 1.1432x; 1.1432x over previous
"""Causal segment-masked depthwise conv (K=5) + pointwise conv, 8-core SPMD.

Strategy (bf16 data path), v2:
  Host: pack each batch row's covered segments into one gap-free global
  stream (T = 32768 = 8 cores x 4096; segments exactly partition [0,L) so
  the stream is x itself), split evenly with a 4-element halo, transpose
  to [C, stream], cast bf16.  Cross-run tap leakage fixed by a sparse
  host-side correction at the first 4 positions of each run.  b_dw folded
  into b_eff = b_pw + w_pw @ b_dw.

  Device per core (4 superblocks of 1024 cols, 4 channel chunks of 128):
  depthwise conv split across engines by measured throughput:
    PE    (diag-matmul, 10 mm/unit @ ~210ns): units (0,2)(0,3)(1,2)(2,0)
          (2,2)(2,3)(3,2)
    DVE   (5 tensor_scalar + 4 tensor_tensor @ ~450/~690ns): units
          (0,0)(1,0)(1,3)(3,0)(3,3), plus tap-scales for the Pool units
    Pool  (4 tensor_tensor adds @ ~2.2us): units (0,1)(1,1)(2,1)
    ACT   tap-scales for (3,1) (adds on DVE), all PSUM evacuations + bias
  Pointwise: per sb 4 dch x [h=0 chain of 4, h=1 chain of 4] accumulating
  matmuls (sequential-h), ACT adds b_eff over 1024 cols + bf16 cast,
  2 batched stores per sb.  Host upcasts + scatters + corrects.
"""

import sys

sys.path.insert(0, "/opt/trn_rl_repo")

import numpy as np
import ml_dtypes

BF16 = ml_dtypes.bfloat16

B, L, C, K, S = 8, 4096, 512, 5, 8
NCORES = 8
CCH = C // 128          # 4 channel chunks
Q = 4096                # stream cols per core
NSB = 4                 # 1024-wide superblocks per core
SBW = 1024
SBPAD = 1032            # 4 halo + 1024 + 4 pad
BLK = 512

# conv unit assignment per (sb, chunk).  Pool is not used: its SBUF
# traffic shares the DVE port and collapses DVE tensor_scalar to ~1/5
# throughput (measured).
PE_UNITS = [(0, 2), (0, 3), (2, 2), (2, 3), (1, 2), (2, 1), (3, 2)]
ACT_UNITS = [(0, 1), (3, 1)]            # scales on ACT, adds on DVE
DVE_UNITS = [(0, 0), (1, 0), (1, 1), (1, 3), (2, 0), (3, 0), (3, 3)]
DIAG_CHUNKS = sorted({j for _, j in PE_UNITS})   # [0, 1, 2, 3]
DIAG_IDX = {j: i for i, j in enumerate(DIAG_CHUNKS)}

_cached = {}


def _build_nc():
    import concourse.mybir as mybir
    from concourse import bacc
    from concourse.tile import TileContext

    f32 = mybir.dt.float32
    bf16 = mybir.dt.bfloat16
    Alu = mybir.AluOpType
    Act = mybir.ActivationFunctionType

    nc = bacc.Bacc(num_swdge_queues=1)
    xin_d = nc.declare_dram_parameter("xin", [NSB, 128, CCH, SBPAD], bf16, isOutput=False)
    # cst: wdiag all 4 chunks (4*K cols) then beff (CCH cols), fp32
    cst_d = nc.declare_dram_parameter("cst", [128, CCH * K + CCH], f32, isOutput=False)
    diag_d = nc.declare_dram_parameter(
        "diag", [128, len(DIAG_CHUNKS), K, 128], bf16, isOutput=False
    )
    wpwt_d = nc.declare_dram_parameter("wpwt", [128, CCH, CCH, 128], bf16, isOutput=False)
    out_d = nc.declare_dram_parameter("out", [NSB, 128, CCH, SBW], bf16, isOutput=True)

    with TileContext(nc) as tc:
        with (
            tc.tile_pool(name="consts", bufs=1) as cpool,
            tc.tile_pool(name="xin", bufs=4) as xin_pool,
            tc.tile_pool(name="dwt", bufs=4) as dwt_pool,
            tc.tile_pool(name="ysc", bufs=3) as y_pool,
            tc.tile_pool(name="outsb", bufs=2) as ob_pool,
            tc.tile_pool(name="dwps", bufs=2, space="PSUM") as dw_psum,
            tc.tile_pool(name="outps", bufs=2, space="PSUM") as out_psum,
            tc.tile_pool(name="p3ps", bufs=2, space="PSUM") as sb3_psum,
        ):
            # scalar ring: cst, diag hi-chunks, xin1, diag lo-chunks, xin3
            # sync ring: xin0, xin2, wpwt (stores also go on sync later)
            cst = cpool.tile([128, CCH * K + CCH], f32)
            nc.scalar.dma_start(out=cst[:], in_=cst_d[:])
            wdiag = cst[:, 0 : CCH * K]
            beff = cst[:, CCH * K : CCH * K + CCH]
            diag = cpool.tile([128, len(DIAG_CHUNKS), K, 128], bf16)
            nc.scalar.dma_start(out=diag[:], in_=diag_d[:])

            xts = [None] * NSB
            t0 = xin_pool.tile([128, CCH, SBPAD], bf16, tag="xin", name="xin0")
            # DVE's chunks (0,1) of sb0 first so it starts earliest
            nc.sync.dma_start(out=t0[:, 0:2], in_=xin_d[0, :, 0:2])
            nc.sync.dma_start(out=t0[:, 2:4], in_=xin_d[0, :, 2:4])
            xts[0] = t0
            t1 = xin_pool.tile([128, CCH, SBPAD], bf16, tag="xin", name="xin1")
            nc.sync.dma_start(out=t1[:], in_=xin_d[1])
            xts[1] = t1
            t2 = xin_pool.tile([128, CCH, SBPAD], bf16, tag="xin", name="xin2")
            nc.scalar.dma_start(out=t2[:], in_=xin_d[2])
            xts[2] = t2
            wpwt = cpool.tile([128, CCH, CCH, 128], bf16)
            nc.sync.dma_start(out=wpwt[:], in_=wpwt_d[:])
            t3 = xin_pool.tile([128, CCH, SBPAD], bf16, tag="xin", name="xin3")
            nc.scalar.dma_start(out=t3[:], in_=xin_d[3])
            xts[3] = t3

            # PE warm-up / filler chains on a memset tile while DMA lands.
            # They use the sb3 psum tag (only needed at the very end) and
            # are never read, so rotation is WAW-only and harmless.
            warm_t = cpool.tile([128, BLK], bf16)
            nc.vector.memset(warm_t[:], 0.0)
            _warm_n = [0]

            def warm(n):
                wp = sb3_psum.tile(
                    [128, BLK], f32, tag="p3", name=f"warm{_warm_n[0]}"
                )
                _warm_n[0] += 1
                for wi in range(n):
                    nc.tensor.matmul(
                        wp[:],
                        lhsT=warm_t[:, 0:128],
                        rhs=warm_t[:],
                        start=(wi == 0),
                        stop=(wi == n - 1),
                    )

            warm(24)

            dwts = [
                [
                    dwt_pool.tile([128, SBW], bf16, tag=f"dwt{j}", name=f"dwt{j}_{sb}")
                    for j in range(CCH)
                ]
                for sb in range(NSB)
            ]

            def conv_pe(sb, j):
                xtv = xts[sb]
                jj = DIAG_IDX[j]
                for h in range(2):
                    ps = dw_psum.tile([128, BLK], f32, tag="dwps", name=f"cps{sb}_{j}_{h}")
                    for k in range(K):
                        nc.tensor.matmul(
                            ps[:],
                            lhsT=diag[:, jj, k, :],
                            rhs=xtv[:, j, h * BLK + k : h * BLK + k + BLK],
                            start=(k == 0),
                            stop=(k == K - 1),
                        )
                    nc.scalar.copy(dwts[sb][j][:, h * BLK : (h + 1) * BLK], ps[:])

            def conv_dve(sb, j):
                # dwt = w0*x[0:1024]; then += wk*x[k:k+1024] via scratch
                xtv = xts[sb]
                sl = dwts[sb][j][:]
                nc.vector.tensor_scalar_mul(
                    sl, xtv[:, j, 0:SBW], wdiag[:, j * K : j * K + 1]
                )
                y = y_pool.tile([128, SBW], bf16, tag="dy", name=f"dy{sb}_{j}")
                for k in range(1, K):
                    nc.vector.tensor_scalar_mul(
                        y[:], xtv[:, j, k : k + SBW], wdiag[:, j * K + k : j * K + k + 1]
                    )
                    nc.vector.tensor_tensor(sl, sl, y[:], op=Alu.add)

            def conv_scales_dve(sb, j):
                # 5 scaled taps into distinct scratch tiles (for Pool adds)
                xtv = xts[sb]
                ys = []
                for k in range(K):
                    y = y_pool.tile([128, SBW], bf16, tag=f"y{k}", name=f"y{k}_{sb}_{j}")
                    nc.vector.tensor_scalar_mul(
                        y[:], xtv[:, j, k : k + SBW], wdiag[:, j * K + k : j * K + k + 1]
                    )
                    ys.append(y)
                return ys

            def conv_scales_act(sb, j):
                xtv = xts[sb]
                ys = []
                for k in range(K):
                    y = y_pool.tile([128, SBW], bf16, tag=f"y{k}", name=f"ya{k}_{sb}_{j}")
                    nc.scalar.activation(
                        y[:], xtv[:, j, k : k + SBW], Act.Copy,
                        bias=0.0, scale=wdiag[:, j * K + k : j * K + k + 1],
                    )
                    ys.append(y)
                return ys

            def conv_adds(engine, sb, j, ys):
                sl = dwts[sb][j][:]
                engine.tensor_tensor(sl, ys[0][:], ys[1][:], op=Alu.add)
                for k in range(2, K):
                    engine.tensor_tensor(sl, sl, ys[k][:], op=Alu.add)

            obs = {}

            def pw_dch(sb, dch):
                dt = dwts[sb]
                if dch == 0:
                    obs[sb] = ob_pool.tile(
                        [128, CCH, SBW], bf16, tag="outsb", name=f"ob{sb}"
                    )
                ob = obs[sb]
                po = out_psum.tile(
                    [128, 2, BLK], f32, tag="outps", name=f"po{dch}_{sb}"
                )
                for h in range(2):
                    for j in range(CCH):
                        nc.tensor.matmul(
                            po[:, h, :],
                            lhsT=wpwt[:, j, dch, :],
                            rhs=dt[j][:, h * BLK : (h + 1) * BLK],
                            start=(j == 0),
                            stop=(j == CCH - 1),
                        )
                nc.scalar.add(ob[:, dch, :], po[:], beff[:, dch : dch + 1])
                # store each dch as soon as it is ready, on the sync ring
                # (idle after the input loads) so the tail drains fast
                nc.sync.dma_start(
                    out=out_d[sb, :, dch : dch + 1, :],
                    in_=ob[:, dch : dch + 1, :],
                )

            # ---- ACT scale-set for (0,1) in ACT's early idle window ----
            ys_01 = conv_scales_act(0, 1)

            # ---- DVE full units (early sbs first, (3,3) before (3,1) adds)
            conv_dve(0, 0)
            conv_dve(1, 0)
            conv_adds(nc.vector, 0, 1, ys_01)
            conv_dve(1, 1)
            conv_dve(1, 3)
            conv_dve(2, 0)
            conv_dve(3, 0)

            # ---- PE conv units, arrival-aware, with filler to absorb
            # DMA-arrival jitter without p-state resets ----
            conv_pe(0, 2)
            conv_pe(0, 3)
            warm(10)
            conv_pe(1, 2)
            warm(8)
            conv_pe(2, 2)
            conv_pe(2, 3)

            # ---- ACT scales for (3,1); DVE finishes sb3 in half-width
            # ops so the pointwise gate lands as early as possible ----
            ys_31 = conv_scales_act(3, 1)

            def conv_dve_half(sb, j, h):
                xtv = xts[sb]
                o = h * BLK
                sl = dwts[sb][j][:, o : o + BLK]
                nc.vector.tensor_scalar_mul(
                    sl, xtv[:, j, o : o + BLK], wdiag[:, j * K : j * K + 1]
                )
                y = y_pool.tile([128, BLK], bf16, tag="dyh", name=f"dyh{sb}_{j}_{h}")
                for k in range(1, K):
                    nc.vector.tensor_scalar_mul(
                        y[:], xtv[:, j, o + k : o + k + BLK],
                        wdiag[:, j * K + k : j * K + k + 1],
                    )
                    nc.vector.tensor_tensor(sl, sl, y[:], op=Alu.add)

            def conv_adds_half(sb, j, ys, h):
                o = h * BLK
                sl = dwts[sb][j][:, o : o + BLK]
                nc.vector.tensor_tensor(
                    sl, ys[0][:, o : o + BLK], ys[1][:, o : o + BLK], op=Alu.add
                )
                for k in range(2, K):
                    nc.vector.tensor_tensor(sl, sl, ys[k][:, o : o + BLK], op=Alu.add)

            conv_dve_half(3, 3, 0)
            conv_adds_half(3, 1, ys_31, 0)
            conv_dve_half(3, 3, 1)
            conv_adds_half(3, 1, ys_31, 1)

            # ---- pointwise; PE order fills DMA/DVE wait with conv units
            conv_pe(2, 1)
            conv_pe(3, 2)
            for dch in range(CCH):
                pw_dch(0, dch)
            for dch in range(CCH):
                pw_dch(1, dch)
            for dch in range(CCH):
                pw_dch(2, dch)
            # sb3 split by halves: h0 runs as soon as the h0 conv lands
            ob3 = ob_pool.tile([128, CCH, SBW], bf16, tag="outsb", name="ob3")
            dt3 = dwts[3]
            for h in range(2):
                for dch in range(CCH):
                    ph = sb3_psum.tile(
                        [128, BLK], f32, tag="p3", name=f"p3_{dch}_{h}"
                    )
                    for j in range(CCH):
                        nc.tensor.matmul(
                            ph[:],
                            lhsT=wpwt[:, j, dch, :],
                            rhs=dt3[j][:, h * BLK : (h + 1) * BLK],
                            start=(j == 0),
                            stop=(j == CCH - 1),
                        )
                    nc.scalar.add(
                        ob3[:, dch, h * BLK : (h + 1) * BLK],
                        ph[:],
                        beff[:, dch : dch + 1],
                    )
                    if h == 1:
                        nc.sync.dma_start(
                            out=out_d[3, :, dch : dch + 1, :],
                            in_=ob3[:, dch : dch + 1, :],
                        )

    nc.finalize()
    return nc


def _get_nc():
    if "nc" not in _cached:
        _cached["nc"] = _build_nc()
    return _cached["nc"]


def _analyze(segment_boundaries):
    starts = segment_boundaries[..., 0].astype(np.int64)  # [B,S]
    ends = segment_boundaries[..., 1].astype(np.int64)
    pos = np.arange(L)
    in_seg = (pos[None, None, :] >= starts[..., None]) & (
        pos[None, None, :] < ends[..., None]
    )  # [B,S,L]
    covered = in_seg.any(axis=1)
    seg_id = np.where(covered, in_seg.argmax(axis=1), -1)  # [B,L]
    return covered, seg_id


def kernel(x, segment_boundaries, w_dw, b_dw, w_pw, b_pw):
    from concourse.bass_utils import run_bass_kernel_spmd

    x = np.asarray(x, dtype=np.float32)
    sb_in = np.asarray(segment_boundaries)
    w_dw = np.asarray(w_dw, dtype=np.float32)
    b_dw = np.asarray(b_dw, dtype=np.float32)
    w_pw = np.asarray(w_pw, dtype=np.float32)
    b_pw = np.asarray(b_pw, dtype=np.float32)

    covered, seg_id = _analyze(sb_in)

    # ---- gap-free run decomposition ----
    runs = []  # (b, s, e, p0) with p0 = stream offset
    pieces = []
    src_b_parts = []
    src_l_parts = []
    p0 = 0
    for b in range(B):
        sid = seg_id[b]
        change = np.nonzero(np.diff(sid) != 0)[0] + 1
        bounds = np.concatenate([[0], change, [L]])
        for s, e in zip(bounds[:-1], bounds[1:]):
            if sid[s] < 0:
                continue
            runs.append((b, int(s), int(e), p0))
            pieces.append(x[b, s:e])
            src_b_parts.append(np.full(e - s, b, np.int64))
            src_l_parts.append(np.arange(s, e, dtype=np.int64))
            p0 += e - s
    if pieces:
        stream = np.concatenate(pieces, axis=0)
        src_b = np.concatenate(src_b_parts)
        src_l = np.concatenate(src_l_parts)
    else:
        stream = np.zeros((0, C), np.float32)
        src_b = np.zeros(0, np.int64)
        src_l = np.zeros(0, np.int64)
    T = stream.shape[0]
    Qc = -(-T // NCORES) if T else 1
    assert Qc <= Q, f"stream quota {Qc} too large"

    # ---- per-core inputs ----
    wdiag = np.ascontiguousarray(
        w_dw.reshape(CCH, 128, K).transpose(1, 0, 2).reshape(128, CCH * K)
    )
    beff_full = b_pw + w_pw @ b_dw                      # [C]
    beffr = np.ascontiguousarray(beff_full.reshape(CCH, 128).T)
    cst = np.concatenate([wdiag, beffr], axis=1).astype(np.float32)
    diag = np.zeros((128, len(DIAG_CHUNKS), K, 128), np.float32)
    for jj, j in enumerate(DIAG_CHUNKS):
        for k in range(K):
            np.fill_diagonal(diag[:, jj, k, :], w_dw[j * 128 : (j + 1) * 128, k])
    diag = diag.astype(BF16)
    wpwt = np.ascontiguousarray(
        w_pw.reshape(CCH, 128, CCH, 128).transpose(3, 2, 0, 1)
    ).astype(BF16)

    # transposed bf16 stream with 4 zero cols in front
    streamT = np.zeros((C, 4 + T), dtype=BF16)
    streamT[:, 4:] = stream.T.astype(BF16)

    in_maps = []
    spans = []
    for i in range(NCORES):
        lo, hi = i * Qc, min((i + 1) * Qc, T)
        lo = min(lo, T)
        spans.append((lo, hi))
        xin = np.zeros((NSB, 128, CCH, SBPAD), dtype=BF16)
        for sbi in range(NSB):
            a = lo + sbi * SBW            # first needed stream col minus 4
            w = min(SBW + 4, 4 + T - a)
            if w <= 0:
                continue
            blkdat = streamT[:, a : a + w]  # [C, w]
            xin[sbi, :, :, :w] = blkdat.reshape(CCH, 128, w).transpose(1, 0, 2)
        in_maps.append({"xin": xin, "cst": cst, "diag": diag, "wpwt": wpwt})

    nc = _get_nc()
    res = run_bass_kernel_spmd(nc, in_maps, list(range(NCORES)))

    # ---- gather (device out is [NSB, 128, CCH, SBW] block-packed) ----
    so_out = np.zeros((T, C), np.float32)
    for i, (lo, hi) in enumerate(spans):
        if hi > lo:
            full = (
                res.results[i]["out"]
                .astype(np.float32)
                .transpose(0, 3, 2, 1)
                .reshape(NSB * SBW, C)
            )
            so_out[lo:hi] = full[: hi - lo]
    out = np.zeros((B, L, C), np.float32)
    out[src_b, src_l] = so_out

    # ---- sparse correction at the first 4 positions of each run ----
    fix_b, fix_l, fix_delta = [], [], []
    for (b, s, e, p0r) in runs:
        n = e - s
        for q in range(min(K - 1, n)):
            l = s + q
            t = p0r + q
            acc = np.zeros(C, np.float32)
            hit = False
            for d in range(q + 1, K):
                v_dev = stream[t - d] if t - d >= 0 else None
                l2 = l - d
                v_ref = (
                    x[b, l2]
                    if (l2 >= 0 and seg_id[b, l2] == seg_id[b, l])
                    else None
                )
                if v_dev is None and v_ref is None:
                    continue
                diff = (v_ref if v_ref is not None else 0.0) - (
                    v_dev if v_dev is not None else 0.0
                )
                acc += w_dw[:, K - 1 - d] * diff
                hit = True
            if hit:
                fix_b.append(b)
                fix_l.append(l)
                fix_delta.append(acc)
    if fix_b:
        deltas = np.stack(fix_delta) @ w_pw.T
        out[np.array(fix_b), np.array(fix_l)] += deltas

    return out
